# revision 24
# baseline (speedup 1.0000x reference)
"""Trainium2 Bass kernel for nn_Attention_36799279792519.

Full causal self-attention layer (QKV proj + RoPE + causal softmax attention +
output proj), B=2 T=2048 C=1024 H=16 D=64, sharded over 8 NeuronCores:
data-parallel on batch (2) x tensor-parallel on heads (4 heads/core).
Each core computes its heads' attention output and a partial projection
(T, C); the host sums the 4 partials per batch and adds proj bias.

v2 layout (per core), built around PE output-column cost:
  x_sb     (128, 8, T)   hidden transposed, C on partitions in 8 chunks
  qk tiles q and k for one PAIR of heads fused in one 128-row tile:
           rows 0-63 = q (2 heads x 32 dims), 64-127 = k. Two tiles per
           pair: "a" = even rotary dims, "b" = odd. RoPE+bias fused into
           PSUM evacuation (scalar_tensor_tensor), then 8 small DMAs
           permute rows into per-head-contiguous qr/kr tiles.
  scores   computed transposed (keys on partitions) per 128-key block x
           512-query tile, both heads of a pair in one PSUM tile; exp'd
           on ACT into persistent `at` tiles; diagonal blocks masked
           multiplicatively with a 0/1 tile (DVE).
  PV       TRANSPOSED: out (queries 128, 65) = at-block^T @ v, with a
           ones column so col 64 accumulates the softmax denominator
           per-partition. Normalization is then a per-partition scalar
           multiply; xbar DMA transposes (q,d)->(d,q) into oT off-PE.
  proj     oT (d_local, T) stationary x wp moving, partial (T, C) out.

Emission is software-pipelined column-block-wise so ACT exp work starts
~5us in and overlaps QKV/V/proj matmuls throughout.

Self-contained: hardcodes all shapes; no sibling imports.
"""
import contextlib

import numpy as np
import ml_dtypes

import concourse.bass as bass
import concourse.mybir as mybir
import concourse.tile as tile
from concourse import bacc
from concourse.bass_utils import run_bass_kernel_spmd

B, T, C = 2, 2048, 1024
H, D = 16, 64
SCALE = D ** -0.5
NCORES = 8
CORES_PER_B = NCORES // B          # 4
HPC = H // CORES_PER_B             # 4 heads per core
RL = HPC * D                       # 256 local q/k/v rows
CCH = C // 128                     # 8 contraction chunks
TCH = T // 128                     # 16 t chunks of 128
NT = T // 512                      # 4 t chunks of 512
KB = T // 128                      # 16 key blocks of 128

F32 = mybir.dt.float32
BF16 = mybir.dt.bfloat16
BF = ml_dtypes.bfloat16

ADD = mybir.AluOpType.add
MULT = mybir.AluOpType.mult

_compiled = {}


def _build():
    nc = bacc.Bacc("TRN2", target_bir_lowering=False, debug=False,
                   num_devices=NCORES)

    d = {}
    d["xT"] = nc.dram_tensor("xT", [C, T], BF16, kind="ExternalInput").ap()
    for p in range(2):
        for ab in range(2):
            d[f"wqk{p}{ab}"] = nc.dram_tensor(
                f"wqk{p}{ab}", [128, CCH * 128], BF16,
                kind="ExternalInput").ap()
            d[f"bqk{p}{ab}"] = nc.dram_tensor(
                f"bqk{p}{ab}", [128], F32, kind="ExternalInput").ap()
    d["wv"] = nc.dram_tensor("wv_t", [C, RL], BF16, kind="ExternalInput").ap()
    d["bv"] = nc.dram_tensor("bv", [RL], F32, kind="ExternalInput").ap()
    d["wp"] = nc.dram_tensor("wproj_t", [RL, C], BF16, kind="ExternalInput").ap()
    d["ck"] = nc.dram_tensor("cos_k", [128, T], BF16, kind="ExternalInput").ap()
    d["sk"] = nc.dram_tensor("sin_k", [128, T], BF16, kind="ExternalInput").ap()
    d["mask"] = nc.dram_tensor("mask01", [128, 128], BF16,
                               kind="ExternalInput").ap()
    d["out"] = nc.dram_tensor("out", [T, C], BF16, kind="ExternalOutput").ap()

    with tile.TileContext(nc) as tc:
        _program(nc, tc, d)

    nc.compile()
    return nc


def _program(nc, tc, d, dbg=None):
    _cm = contextlib.ExitStack()
    _cmx = contextlib.ExitStack()
    with (
        tc.tile_pool(name="const", bufs=1) as const,
        tc.tile_pool(name="qk", bufs=1) as qkpool,
        tc.tile_pool(name="atp", bufs=1) as atpool,
        tc.tile_pool(name="work", bufs=2) as wpool,
        tc.tile_pool(name="small", bufs=4) as spool,
        tc.tile_pool(name="outsb", bufs=4) as opool,
        tc.tile_pool(name="ps_pv", bufs=2, space="PSUM") as ps_pv,
        _cm,
        _cmx,
    ):
        _cmsc = contextlib.ExitStack()
        ps_sc = _cmsc.enter_context(
            tc.tile_pool(name="ps_sc", bufs=1, space="PSUM"))
        ps_qkv = _cm.enter_context(
            tc.tile_pool(name="ps_qkv", bufs=2, space="PSUM"))
        xpool = _cmx.enter_context(tc.tile_pool(name="xpool", bufs=1))
        _cmw = contextlib.ExitStack()
        wqpool = _cmw.enter_context(tc.tile_pool(name="wqpool", bufs=1))

        # ================= long-lived tiles =================
        wp_sb = const.tile([128, 2, C], BF16)
        mask_sb = const.tile([128, 128], BF16)
        x_sb = xpool.tile([128, CCH, T], BF16)
        ck_sb = wqpool.tile([128, T], BF16)
        sk_sb = wqpool.tile([128, T], BF16)
        wqk_sb = [[wqpool.tile([128, CCH, 128], BF16, name=f"wqk{p}{ab}",
                               tag=f"wqk{p}{ab}") for ab in range(2)]
                  for p in range(2)]
        bqk_sb = [[wqpool.tile([128, 1], F32, name=f"bqk{p}{ab}",
                              tag=f"bqk{p}{ab}") for ab in range(2)]
                  for p in range(2)]
        wv_sb = xpool.tile([128, CCH, RL], BF16)
        bv_bc = const.tile([128, RL], F32)

        rot_a = wqpool.tile([128, T], BF16, tag="rota", name="rot_a")
        rot_b = wqpool.tile([128, T], BF16, tag="rotb", name="rot_b")
        v_sb = qkpool.tile([128, KB, HPC, 65], BF16, tag="v")
        nc.vector.memset(v_sb[:, :, :, 64:65], 1.0)
        qr = [qkpool.tile([128, T], BF16, tag=f"qr{p}", name=f"qr{p}")
              for p in range(2)]
        kr = [qkpool.tile([128, T], BF16, tag=f"kr{p}", name=f"kr{p}")
              for p in range(2)]
        oT = [qkpool.tile([128, T], BF16, tag=f"oT{p}", name=f"oT{p}")
              for p in range(2)]

        # ================= input DMAs (pair-0 weights + x first) ========
        xT_r = d["xT"].rearrange("(cc p) t -> p cc t", p=128)

        def load_wqk(p):
            for ab in range(2):
                nc.sync.dma_start(
                    out=wqk_sb[p][ab],
                    in_=d[f"wqk{p}{ab}"].rearrange("p (cc r) -> p cc r", r=128))
                nc.sync.dma_start(
                    out=bqk_sb[p][ab],
                    in_=d[f"bqk{p}{ab}"].rearrange("(p one) -> p one", one=1))

        nc.sync.dma_start(
            out=wqk_sb[0][0],
            in_=d["wqk00"].rearrange("p (cc r) -> p cc r", r=128))
        nc.sync.dma_start(out=x_sb[:, :, 0:512], in_=xT_r[:, :, 0:512])
        nc.sync.dma_start(
            out=wqk_sb[0][1],
            in_=d["wqk01"].rearrange("p (cc r) -> p cc r", r=128))
        for ab in range(2):
            nc.sync.dma_start(
                out=bqk_sb[0][ab],
                in_=d[f"bqk0{ab}"].rearrange("(p one) -> p one", one=1))
        nc.sync.dma_start(out=ck_sb, in_=d["ck"])
        nc.sync.dma_start(out=sk_sb, in_=d["sk"])
        for q in range(1, 4):
            nc.sync.dma_start(out=x_sb[:, :, q * 512:(q + 1) * 512],
                              in_=xT_r[:, :, q * 512:(q + 1) * 512])
        load_wqk(1)  # pair-1 weights
        nc.sync.dma_start(out=wv_sb,
                          in_=d["wv"].rearrange("(cc p) r -> p cc r", p=128))
        nc.sync.dma_start(
            out=bv_bc,
            in_=bass.AP(tensor=d["bv"].tensor, offset=d["bv"].offset,
                        ap=[[0, 128]] + list(d["bv"].ap)))
        nc.sync.dma_start(out=mask_sb, in_=d["mask"])
        nc.sync.dma_start(out=wp_sb,
                          in_=d["wp"].rearrange("(dc p) c -> p dc c", p=128))

        # =========== step emitters (PE order == emission order) =========
        def qk_step(p, tc_):
            """QKV matmuls for pair p's (q|k) tile, t-slice tc_, with fused
            bias + RoPE during PSUM evacuation, then row-permute DMAs."""
            sl = slice(tc_ * 512, (tc_ + 1) * 512)
            ps = []
            for ab in range(2):
                t_ = ps_qkv.tile([128, 512], F32, tag="qkv", name="psqkv")
                for i in range(CCH):
                    cc = (i + 2 * tc_ + ab) % CCH
                    nc.tensor.matmul(t_, wqk_sb[p][ab][:, cc, :],
                                     x_sb[:, cc, sl],
                                     start=(i == 0), stop=(i == CCH - 1))
                ps.append(t_)
            c_sl, s_sl = ck_sb[:, sl], sk_sb[:, sl]
            t1 = wqpool.tile([128, 512], BF16, tag="t1", name="t1", bufs=2)
            t2 = wqpool.tile([128, 512], BF16, tag="t2", name="t2", bufs=2)
            nc.vector.scalar_tensor_tensor(out=t1, in0=ps[0],
                                           scalar=bqk_sb[p][0], in1=c_sl,
                                           op0=ADD, op1=MULT)
            nc.vector.scalar_tensor_tensor(out=t2, in0=ps[1],
                                           scalar=bqk_sb[p][1], in1=s_sl,
                                           op0=ADD, op1=MULT)
            nc.vector.tensor_sub(rot_a[:, sl], t1, t2)
            t3 = wqpool.tile([128, 512], BF16, tag="t1", name="t3", bufs=2)
            t4 = wqpool.tile([128, 512], BF16, tag="t2", name="t4", bufs=2)
            nc.vector.scalar_tensor_tensor(out=t3, in0=ps[0],
                                           scalar=bqk_sb[p][0], in1=s_sl,
                                           op0=ADD, op1=MULT)
            nc.vector.scalar_tensor_tensor(out=t4, in0=ps[1],
                                           scalar=bqk_sb[p][1], in1=c_sl,
                                           op0=ADD, op1=MULT)
            nc.vector.tensor_add(rot_b[:, sl], t3, t4)

        def permute(p, c0, c1):
            # rows: rot_a = [q h0 ev | q h1 ev | k h0 ev | k h1 ev] (32 each)
            # dst per-head layout: [32 rot-ev ; 32 rot-od]
            # issued on the ACT hwdge queue: idle early, bypasses the SP
            # input-load queue in the scheduler's readiness model
            for hh in range(2):
                for half, src in ((0, rot_a), (1, rot_b)):
                    r0 = hh * 64 + half * 32
                    nc.sync.dma_start(out=qr[p][r0:r0 + 32, c0:c1],
                                      in_=src[hh * 32:(hh + 1) * 32, c0:c1])
                    nc.sync.dma_start(out=kr[p][r0:r0 + 32, c0:c1],
                                      in_=src[64 + hh * 32:64 + (hh + 1) * 32, c0:c1])

        def v_step(kc):
            ps = ps_qkv.tile([128, 512], F32, tag="qkv", name="psv")
            psv = ps[:, 0:RL]
            for i in range(CCH):
                cc = (i + kc) % CCH
                nc.tensor.matmul(
                    psv, x_sb[:, cc, kc * 128:(kc + 1) * 128], wv_sb[:, cc, :],
                    start=(i == 0), stop=(i == CCH - 1))
            nc.vector.scalar_tensor_tensor(
                out=v_sb[:, kc, :, 0:64],
                in0=psv.rearrange("p (h dd) -> p h dd", h=HPC),
                scalar=0.0,
                in1=bv_bc.rearrange("p (h dd) -> p h dd", h=HPC),
                op0=ADD, op1=ADD)

        # at storage: kb-PAIR tiles [128, 2(kb), 2(h), T - kbp*256] so one
        # exp instruction can cover both kbs of an off-diagonal pair
        atp_tiles = [{} for _ in range(2)]  # per pair: kbp -> tile

        def _at_tile(p, kbp):
            if kbp not in atp_tiles[p]:
                if p == 0 or kbp >= 6:
                    pool, tg = atpool, f"at{kbp}"
                else:
                    pool, tg = atp1[0], f"at1_{kbp}"
                atp_tiles[p][kbp] = pool.tile(
                    [128, 2, 2, T - kbp * 256], BF16,
                    tag=tg, name=f"at{p}_{kbp}")
            return atp_tiles[p][kbp]

        def sc_group(p, tau, g):
            """Scores for the kbs in group g (1 or 2 kbs) at query tile tau,
            exp'd into the kb-pair tile; diagonal blocks get the 0/1 mask."""
            kbp = g[0] // 2
            at2 = _at_tile(p, kbp)
            ps = ps_sc.tile([128, 2, 2, 512], F32, tag="sc", name="ps_sc")
            offs = []
            for kb in g:
                k0 = kb * 128
                off = max(0, k0 - tau * 512)
                offs.append(off)
                qsl = slice(tau * 512 + off, (tau + 1) * 512)
                for h in range(2):
                    nc.tensor.matmul(ps[:, kb % 2, h, off:512],
                                     kr[p][h * 64:(h + 1) * 64, k0:k0 + 128],
                                     qr[p][h * 64:(h + 1) * 64, qsl],
                                     start=True, stop=True)
            if len(g) == 2 and offs == [0, 0]:
                pos = tau * 512 - kbp * 256
                nc.scalar.activation(out=at2[:, :, :, pos:pos + 512],
                                     in_=ps,
                                     func=mybir.ActivationFunctionType.Exp)
            else:
                for kb in g:
                    k0 = kb * 128
                    off = max(0, k0 - tau * 512)
                    pos = tau * 512 + off - k0
                    nc.scalar.activation(
                        out=at2[:, kb % 2, :, k0 - kbp * 256 + pos:
                                k0 - kbp * 256 + pos + 512 - off],
                        in_=ps[:, kb % 2, :, off:512],
                        func=mybir.ActivationFunctionType.Exp)
            for kb in g:
                if tau == kb // 4:
                    pos0 = kb * 128 - kbp * 256
                    for h in range(2):
                        nc.vector.tensor_mul(
                            at2[:, kb % 2, h, pos0:pos0 + 128],
                            at2[:, kb % 2, h, pos0:pos0 + 128], mask_sb)

        def att_sc(p, tau):
            with tc.high_priority():
                groups = ([[2 * i, 2 * i + 1] for i in range(2 * tau)]
                          + [[kb] for kb in range(4 * tau, 4 * tau + 4)])
                for g in groups:
                    sc_group(p, tau, g)

        def pv_step(p, qb):
            """Transposed PV for query block qb: psum (128 q, 65) per head;
            col 64 = denominator. Normalize per-partition, then xbar-DMA
            transpose (q,(h,d)) -> ((h,d),q) into oT."""
            pvps = ps_pv.tile([128, 2, 65], F32, tag="pv", name="ps_pv")
            for h in range(2):
                for kb in range(qb + 1):
                    c0 = qb * 128 - (kb // 2) * 256
                    nc.tensor.matmul(pvps[:, h, :],
                                     atp_tiles[p][kb // 2][:, kb % 2, h,
                                                           c0:c0 + 128],
                                     v_sb[:, kb, 2 * p + h, :],
                                     start=(kb == 0), stop=(kb == qb))
            o_sb = spool.tile([128, 128], BF16, tag="osb", name="osb")
            rec = spool.tile([128, 2], F32, tag="rec", name="rec")
            nc.vector.reciprocal(rec, pvps[:, :, 64:65].rearrange("p a b -> p (a b)"))
            for h in range(2):
                nc.vector.tensor_scalar_mul(o_sb[:, h * 64:(h + 1) * 64],
                                            pvps[:, h, 0:64], rec[:, h:h + 1])
            nc.sync.dma_start_transpose(
                out=oT[p][:, qb * 128:(qb + 1) * 128], in_=o_sb)

        def att_pv(p, tau):
            for qb in range(4 * tau, 4 * tau + 4):
                pv_step(p, qb)

        def proj_step(t16, ps_proj):
            o_out = opool.tile([128, C], BF16, tag="oout", name="oout")
            ps = ps_proj.tile([128, C], F32, tag="proj", name="psproj")
            for half in range(2):
                for dc in range(2):
                    nc.tensor.matmul(
                        ps[:, half * 512:(half + 1) * 512],
                        oT[dc][:, t16 * 128:(t16 + 1) * 128],
                        wp_sb[:, dc, half * 512:(half + 1) * 512],
                        start=(dc == 0), stop=(dc == 1))
            if t16 % 2 == 0:
                nc.vector.tensor_copy(o_out, ps)
            else:
                nc.scalar.copy(o_out, ps)
            nc.sync.dma_start(out=d["out"][t16 * 128:(t16 + 1) * 128, :],
                              in_=o_out)

        # ==================== pipelined emission ====================
        atp1 = [None]
        qk_step(0, 0)
        permute(0, 0, 512)
        qk_step(0, 1)
        permute(0, 512, 1024)
        att_sc(0, 0)
        qk_step(0, 2)
        att_sc(0, 1)
        qk_step(0, 3)
        permute(0, 1024, 2048)
        import os as _os
        _dq = float(_os.environ.get("D_QK1", "30")) / 1000.0
        _dv = float(_os.environ.get("D_V", "44")) / 1000.0
        with tc.tile_wait_until(_dq):
            qk_step(1, 0)
        att_sc(0, 2)
        with tc.tile_wait_until(_dq + 0.004):
            qk_step(1, 1)
        att_sc(0, 3)
        with tc.tile_wait_until(_dq + 0.008):
            qk_step(1, 2)
            qk_step(1, 3)
        permute(1, 0, 2048)
        _cmw.close()          # ck/sk/wqk/rot dead
        with tc.tile_wait_until(_dv):
            for kc in range(0, 16):
                v_step(kc)

        # x and the qkv psum are dead; free for pair-1 at tiles + proj psum
        _cm.close()
        _cmx.close()
        atp1[0] = _cm.enter_context(tc.tile_pool(name="atp1", bufs=1))

        att_sc(1, 0)          # atp1 tags, reuse x region
        att_sc(1, 1)
        att_pv(0, 0)
        att_pv(0, 1)
        att_sc(1, 2)          # atp1 tags too -> independent of pair-0 pv
        att_pv(0, 2)
        att_pv(0, 3)
        att_pv(1, 0)
        att_sc(1, 3)          # shared tags (WAR-safe after att_pv(0, 3))
        _cmsc.close()         # scores psum banks -> proj
        ps_proj = _cm.enter_context(
            tc.tile_pool(name="ps_proj", bufs=3, space="PSUM"))
        att_pv(1, 1)
        for t16 in range(0, 8):
            proj_step(t16, ps_proj)
        att_pv(1, 2)
        for t16 in range(8, 12):
            proj_step(t16, ps_proj)
        att_pv(1, 3)
        for t16 in range(12, 16):
            proj_step(t16, ps_proj)

        if dbg is not None:
            nc.sync.dma_start(out=dbg["qr0"], in_=qr[0])
            nc.sync.dma_start(out=dbg["kr0"], in_=kr[0])
            nc.sync.dma_start(out=dbg["v"],
                              in_=v_sb.rearrange("p a b c -> p (a b c)"))
            nc.sync.dma_start(
                out=dbg["at0"],
                in_=atp_tiles[1][0][:, 0, :, :].rearrange("p a b -> p (a b)"))
            nc.sync.dma_start(
                out=dbg["at5"],
                in_=atp_tiles[1][2][:, 1, :, :].rearrange("p a b -> p (a b)"))
            nc.sync.dma_start(out=dbg["oT0"], in_=oT[0])
            nc.sync.dma_start(out=dbg["oT1"], in_=oT[1])


def _host_prep(hidden_states, cos, sin, qkv_w, qkv_b, proj_w):
    cos_rep = np.tile(np.ascontiguousarray(cos.T), (4, 1))
    sin_rep = np.tile(np.ascontiguousarray(sin.T), (4, 1))
    ck = cos_rep.astype(BF)
    sk = sin_rep.astype(BF)
    mask01 = (np.arange(128)[:, None] <= np.arange(128)[None, :]).astype(BF)

    in_maps = []
    for c in range(NCORES):
        b = c // CORES_PER_B
        h0 = (c % CORES_PER_B) * HPC
        heads = list(range(h0, h0 + HPC))
        vrows = [h * D + dd for h in heads for dd in range(D)]
        m = dict(
            xT=np.ascontiguousarray(hidden_states[b].T).astype(BF),
            wv_t=np.ascontiguousarray(
                qkv_w[2 * H * D:3 * H * D][vrows].T).astype(BF),
            bv=np.ascontiguousarray(qkv_b[2 * H * D:3 * H * D][vrows]),
            wproj_t=np.ascontiguousarray(proj_w[:, vrows].T).astype(BF),
            cos_k=ck, sin_k=sk, mask01=mask01,
        )
        qw = qkv_w[0 * H * D:1 * H * D]
        kw = qkv_w[1 * H * D:2 * H * D]
        qb_ = qkv_b[0 * H * D:1 * H * D]
        kb_ = qkv_b[1 * H * D:2 * H * D]
        for p in range(2):
            hA, hB = h0 + 2 * p, h0 + 2 * p + 1
            for ab in range(2):
                # rows: q-hA dims, q-hB dims, k-hA dims, k-hB dims (32 each),
                # dims = even (ab=0) or odd (ab=1) rotary positions
                dims = [2 * j + ab for j in range(D // 2)]
                rows_q = [hA * D + dd for dd in dims] + \
                         [hB * D + dd for dd in dims]
                rows_k = rows_q
                wtile = np.concatenate(
                    [qw[rows_q] * SCALE, kw[rows_k]], axis=0)   # (128, C)
                btile = np.concatenate(
                    [qb_[rows_q] * SCALE, kb_[rows_k]], axis=0)  # (128,)
                wt = wtile.T.reshape(CCH, 128, 128).transpose(1, 0, 2)
                m[f"wqk{p}{ab}"] = np.ascontiguousarray(
                    wt.reshape(128, CCH * 128)).astype(BF)
                m[f"bqk{p}{ab}"] = np.ascontiguousarray(btile)
        in_maps.append(m)
    return in_maps


def kernel(hidden_states, cos, sin, qkv_w, qkv_b, proj_w, proj_b):
    hidden_states = np.asarray(hidden_states, dtype=np.float32)
    cos = np.asarray(cos, dtype=np.float32)
    sin = np.asarray(sin, dtype=np.float32)
    qkv_w = np.asarray(qkv_w, dtype=np.float32)
    qkv_b = np.asarray(qkv_b, dtype=np.float32)
    proj_w = np.asarray(proj_w, dtype=np.float32)
    proj_b = np.asarray(proj_b, dtype=np.float32)

    if "nc" not in _compiled:
        _compiled["nc"] = _build()
    nc = _compiled["nc"]

    in_maps = _host_prep(hidden_states, cos, sin, qkv_w, qkv_b, proj_w)
    res = run_bass_kernel_spmd(nc, in_maps, core_ids=list(range(NCORES)))
    outs = [np.asarray(res.results[c]["out"], dtype=np.float32)
            for c in range(NCORES)]
    final = np.empty((B, T, C), np.float32)
    for b in range(B):
        acc = outs[b * CORES_PER_B].copy()
        for i in range(1, CORES_PER_B):
            acc += outs[b * CORES_PER_B + i]
        final[b] = acc + proj_b[None, :]
    return final


# revision 25
# speedup vs baseline: 1.2440x; 1.2440x over previous
"""Trainium2 Bass kernel for nn_Attention_36799279792519.

Full causal self-attention layer (QKV proj + RoPE + causal softmax attention +
output proj), B=2 T=2048 C=1024 H=16 D=64, sharded over 8 NeuronCores:
data-parallel on batch (2) x tensor-parallel on heads (4 heads/core).
Each core computes its heads' attention output and a partial projection
(T, C); the host sums the 4 partials per batch and adds proj bias.

v2 layout (per core), built around PE output-column cost:
  x_sb     (128, 8, T)   hidden transposed, C on partitions in 8 chunks
  qk tiles q and k for one PAIR of heads fused in one 128-row tile:
           rows 0-63 = q (2 heads x 32 dims), 64-127 = k. Two tiles per
           pair: "a" = even rotary dims, "b" = odd. RoPE+bias fused into
           PSUM evacuation (scalar_tensor_tensor), then 8 small DMAs
           permute rows into per-head-contiguous qr/kr tiles.
  scores   computed transposed (keys on partitions) per 128-key block x
           512-query tile, both heads of a pair in one PSUM tile; exp'd
           on ACT into persistent `at` tiles; diagonal blocks masked
           multiplicatively with a 0/1 tile (DVE).
  PV       TRANSPOSED: out (queries 128, 65) = at-block^T @ v, with a
           ones column so col 64 accumulates the softmax denominator
           per-partition. Normalization is then a per-partition scalar
           multiply; xbar DMA transposes (q,d)->(d,q) into oT off-PE.
  proj     oT (d_local, T) stationary x wp moving, partial (T, C) out.

Emission is software-pipelined column-block-wise so ACT exp work starts
~5us in and overlaps QKV/V/proj matmuls throughout.

Self-contained: hardcodes all shapes; no sibling imports.
"""
import contextlib

import numpy as np
import ml_dtypes

import concourse.bass as bass
import concourse.mybir as mybir
import concourse.tile as tile
from concourse import bacc
from concourse.bass_utils import run_bass_kernel_spmd

B, T, C = 2, 2048, 1024
H, D = 16, 64
SCALE = D ** -0.5
NCORES = 8
CORES_PER_B = NCORES // B          # 4
HPC = H // CORES_PER_B             # 4 heads per core
RL = HPC * D                       # 256 local q/k/v rows
CCH = C // 128                     # 8 contraction chunks
TCH = T // 128                     # 16 t chunks of 128
NT = T // 512                      # 4 t chunks of 512
KB = T // 128                      # 16 key blocks of 128

F32 = mybir.dt.float32
BF16 = mybir.dt.bfloat16
BF = ml_dtypes.bfloat16

ADD = mybir.AluOpType.add
MULT = mybir.AluOpType.mult

_compiled = {}


def _build():
    nc = bacc.Bacc("TRN2", target_bir_lowering=False, debug=False,
                   num_devices=NCORES)

    d = {}
    d["xT"] = nc.dram_tensor("xT", [C, T], BF16, kind="ExternalInput").ap()
    for p in range(2):
        for ab in range(2):
            d[f"wqk{p}{ab}"] = nc.dram_tensor(
                f"wqk{p}{ab}", [128, CCH * 128], BF16,
                kind="ExternalInput").ap()
            d[f"bqk{p}{ab}"] = nc.dram_tensor(
                f"bqk{p}{ab}", [128], F32, kind="ExternalInput").ap()
    d["wv"] = nc.dram_tensor("wv_t", [C, RL], BF16, kind="ExternalInput").ap()
    d["bv"] = nc.dram_tensor("bv", [RL], F32, kind="ExternalInput").ap()
    d["wp"] = nc.dram_tensor("wproj_t", [RL, C], BF16, kind="ExternalInput").ap()
    d["ck"] = nc.dram_tensor("cos_k", [128, T], BF16, kind="ExternalInput").ap()
    d["sk"] = nc.dram_tensor("sin_k", [128, T], BF16, kind="ExternalInput").ap()
    d["mask"] = nc.dram_tensor("mask01", [128, 128], BF16,
                               kind="ExternalInput").ap()
    d["out"] = nc.dram_tensor("out", [T, C], BF16, kind="ExternalOutput").ap()

    with tile.TileContext(nc) as tc:
        _program(nc, tc, d)

    nc.compile()
    return nc


def _program(nc, tc, d, dbg=None):
    _cm = contextlib.ExitStack()
    _cmx = contextlib.ExitStack()
    with (
        tc.tile_pool(name="const", bufs=1) as const,
        tc.tile_pool(name="qk", bufs=1) as qkpool,
        tc.tile_pool(name="atp", bufs=1) as atpool,
        tc.tile_pool(name="work", bufs=2) as wpool,
        tc.tile_pool(name="small", bufs=4) as spool,
        tc.tile_pool(name="outsb", bufs=4) as opool,
        tc.tile_pool(name="ps_pv", bufs=2, space="PSUM") as ps_pv,
        _cm,
        _cmx,
    ):
        _cmsc = contextlib.ExitStack()
        ps_sc = _cmsc.enter_context(
            tc.tile_pool(name="ps_sc", bufs=2, space="PSUM"))
        ps_qkv = _cm.enter_context(
            tc.tile_pool(name="ps_qkv", bufs=2, space="PSUM"))
        xpool = _cmx.enter_context(tc.tile_pool(name="xpool", bufs=1))
        _cmw = contextlib.ExitStack()
        wqpool = _cmw.enter_context(tc.tile_pool(name="wqpool", bufs=1))

        # ================= long-lived tiles =================
        wp_sb = const.tile([128, 2, C], BF16)
        mask_sb = const.tile([128, 128], BF16)
        x_sb = xpool.tile([128, CCH, T], BF16)
        ck_sb = wqpool.tile([128, T], BF16)
        sk_sb = wqpool.tile([128, T], BF16)
        wqk_sb = [[wqpool.tile([128, CCH, 128], BF16, name=f"wqk{p}{ab}",
                               tag=f"wqk{p}{ab}") for ab in range(2)]
                  for p in range(2)]
        bqk_sb = [[wqpool.tile([128, 1], F32, name=f"bqk{p}{ab}",
                              tag=f"bqk{p}{ab}") for ab in range(2)]
                  for p in range(2)]
        wv_sb = xpool.tile([128, CCH, RL], BF16)
        bv_bc = const.tile([128, RL], F32)

        rot_a = wqpool.tile([128, T], BF16, tag="rota", name="rot_a")
        rot_b = wqpool.tile([128, T], BF16, tag="rotb", name="rot_b")
        v_sb = qkpool.tile([128, KB, HPC, 65], BF16, tag="v")
        nc.vector.memset(v_sb[:, :, :, 64:65], 1.0)
        qr = [qkpool.tile([128, T], BF16, tag=f"qr{p}", name=f"qr{p}")
              for p in range(2)]
        kr = [qkpool.tile([128, T], BF16, tag=f"kr{p}", name=f"kr{p}")
              for p in range(2)]
        oT = [qkpool.tile([128, T], BF16, tag=f"oT{p}", name=f"oT{p}")
              for p in range(2)]

        # ================= input DMAs (pair-0 weights + x first) ========
        xT_r = d["xT"].rearrange("(cc p) t -> p cc t", p=128)

        def load_wqk(p):
            for ab in range(2):
                nc.sync.dma_start(
                    out=wqk_sb[p][ab],
                    in_=d[f"wqk{p}{ab}"].rearrange("p (cc r) -> p cc r", r=128))
                nc.sync.dma_start(
                    out=bqk_sb[p][ab],
                    in_=d[f"bqk{p}{ab}"].rearrange("(p one) -> p one", one=1))

        nc.sync.dma_start(
            out=wqk_sb[0][0],
            in_=d["wqk00"].rearrange("p (cc r) -> p cc r", r=128))
        nc.sync.dma_start(out=x_sb[:, :, 0:512], in_=xT_r[:, :, 0:512])
        nc.sync.dma_start(
            out=wqk_sb[0][1],
            in_=d["wqk01"].rearrange("p (cc r) -> p cc r", r=128))
        for ab in range(2):
            nc.sync.dma_start(
                out=bqk_sb[0][ab],
                in_=d[f"bqk0{ab}"].rearrange("(p one) -> p one", one=1))
        nc.sync.dma_start(out=ck_sb, in_=d["ck"])
        nc.sync.dma_start(out=sk_sb, in_=d["sk"])
        for q in range(1, 4):
            nc.sync.dma_start(out=x_sb[:, :, q * 512:(q + 1) * 512],
                              in_=xT_r[:, :, q * 512:(q + 1) * 512])
        load_wqk(1)  # pair-1 weights
        nc.sync.dma_start(out=wv_sb,
                          in_=d["wv"].rearrange("(cc p) r -> p cc r", p=128))
        nc.sync.dma_start(
            out=bv_bc,
            in_=bass.AP(tensor=d["bv"].tensor, offset=d["bv"].offset,
                        ap=[[0, 128]] + list(d["bv"].ap)))
        nc.sync.dma_start(out=mask_sb, in_=d["mask"])
        nc.sync.dma_start(out=wp_sb,
                          in_=d["wp"].rearrange("(dc p) c -> p dc c", p=128))

        # =========== step emitters (PE order == emission order) =========
        def qk_step(p, tc_):
            """QKV matmuls for pair p's (q|k) tile, t-slice tc_, with fused
            bias + RoPE during PSUM evacuation, then row-permute DMAs."""
            sl = slice(tc_ * 512, (tc_ + 1) * 512)
            ps = []
            for ab in range(2):
                t_ = ps_qkv.tile([128, 512], F32, tag="qkv", name="psqkv")
                for i in range(CCH):
                    cc = (i + 2 * tc_ + ab) % CCH
                    nc.tensor.matmul(t_, wqk_sb[p][ab][:, cc, :],
                                     x_sb[:, cc, sl],
                                     start=(i == 0), stop=(i == CCH - 1))
                ps.append(t_)
            c_sl, s_sl = ck_sb[:, sl], sk_sb[:, sl]
            t1 = wqpool.tile([128, 512], BF16, tag="t1", name="t1", bufs=2)
            t2 = wqpool.tile([128, 512], BF16, tag="t2", name="t2", bufs=2)
            nc.vector.scalar_tensor_tensor(out=t1, in0=ps[0],
                                           scalar=bqk_sb[p][0], in1=c_sl,
                                           op0=ADD, op1=MULT)
            nc.vector.scalar_tensor_tensor(out=t2, in0=ps[1],
                                           scalar=bqk_sb[p][1], in1=s_sl,
                                           op0=ADD, op1=MULT)
            nc.vector.tensor_sub(rot_a[:, sl], t1, t2)
            t3 = wqpool.tile([128, 512], BF16, tag="t1", name="t3", bufs=2)
            t4 = wqpool.tile([128, 512], BF16, tag="t2", name="t4", bufs=2)
            nc.vector.scalar_tensor_tensor(out=t3, in0=ps[0],
                                           scalar=bqk_sb[p][0], in1=s_sl,
                                           op0=ADD, op1=MULT)
            nc.vector.scalar_tensor_tensor(out=t4, in0=ps[1],
                                           scalar=bqk_sb[p][1], in1=c_sl,
                                           op0=ADD, op1=MULT)
            nc.vector.tensor_add(rot_b[:, sl], t3, t4)

        def permute(p, c0, c1):
            # rows: rot_a = [q h0 ev | q h1 ev | k h0 ev | k h1 ev] (32 each)
            # dst per-head layout: [32 rot-ev ; 32 rot-od]
            # issued on the ACT hwdge queue: idle early, bypasses the SP
            # input-load queue in the scheduler's readiness model
            for hh in range(2):
                for half, src in ((0, rot_a), (1, rot_b)):
                    r0 = hh * 64 + half * 32
                    nc.sync.dma_start(out=qr[p][r0:r0 + 32, c0:c1],
                                      in_=src[hh * 32:(hh + 1) * 32, c0:c1])
                    nc.sync.dma_start(out=kr[p][r0:r0 + 32, c0:c1],
                                      in_=src[64 + hh * 32:64 + (hh + 1) * 32, c0:c1])

        def v_step(kc):
            ps = ps_qkv.tile([128, 512], F32, tag="qkv", name="psv")
            psv = ps[:, 0:RL]
            for i in range(CCH):
                cc = (i + kc) % CCH
                nc.tensor.matmul(
                    psv, x_sb[:, cc, kc * 128:(kc + 1) * 128], wv_sb[:, cc, :],
                    start=(i == 0), stop=(i == CCH - 1))
            nc.vector.scalar_tensor_tensor(
                out=v_sb[:, kc, :, 0:64],
                in0=psv.rearrange("p (h dd) -> p h dd", h=HPC),
                scalar=0.0,
                in1=bv_bc.rearrange("p (h dd) -> p h dd", h=HPC),
                op0=ADD, op1=ADD)

        # at storage: kb-PAIR tiles [128, 2(kb), 2(h), T - kbp*256] so one
        # exp instruction can cover both kbs of an off-diagonal pair
        atp_tiles = [{} for _ in range(2)]  # per pair: kbp -> tile

        def _at_tile(p, kbp):
            if kbp not in atp_tiles[p]:
                if p == 0 or kbp >= 6:
                    pool, tg = atpool, f"at{kbp}"
                else:
                    pool, tg = atp1[0], f"at1_{kbp}"
                atp_tiles[p][kbp] = pool.tile(
                    [128, 2, 2, T - kbp * 256], BF16,
                    tag=tg, name=f"at{p}_{kbp}")
            return atp_tiles[p][kbp]

        def sc_step(p, tau, kb):
            """Scores for key block kb at query tile tau (both heads), exp'd
            into the kb-pair at tile; diagonal blocks get the 0/1 mask."""
            kbp = kb // 2
            at2 = _at_tile(p, kbp)
            k0 = kb * 128
            off = max(0, k0 - tau * 512)
            ps = ps_sc.tile([128, 2, 512], F32, tag="sc", name="ps_sc")
            qsl = slice(tau * 512 + off, (tau + 1) * 512)
            for h in range(2):
                nc.tensor.matmul(ps[:, h, off:512],
                                 kr[p][h * 64:(h + 1) * 64, k0:k0 + 128],
                                 qr[p][h * 64:(h + 1) * 64, qsl],
                                 start=True, stop=True)
            pos = tau * 512 + off - kbp * 256
            nc.scalar.activation(out=at2[:, kb % 2, :, pos:pos + 512 - off],
                                 in_=ps[:, :, off:512],
                                 func=mybir.ActivationFunctionType.Exp)
            if tau == kb // 4:
                pos0 = k0 - kbp * 256
                for h in range(2):
                    nc.vector.tensor_mul(
                        at2[:, kb % 2, h, pos0:pos0 + 128],
                        at2[:, kb % 2, h, pos0:pos0 + 128], mask_sb)

        def att_sc(p, tau):
            with tc.high_priority():
                for kb in range(4 * tau + 4):
                    sc_step(p, tau, kb)

        def pv_step(p, qb):
            """Transposed PV for query block qb: psum (128 q, 65) per head;
            col 64 = denominator. Normalize per-partition, then xbar-DMA
            transpose (q,(h,d)) -> ((h,d),q) into oT."""
            pvps = ps_pv.tile([128, 2, 65], F32, tag="pv", name="ps_pv")
            for h in range(2):
                for kb in range(qb + 1):
                    c0 = qb * 128 - (kb // 2) * 256
                    nc.tensor.matmul(pvps[:, h, :],
                                     atp_tiles[p][kb // 2][:, kb % 2, h,
                                                           c0:c0 + 128],
                                     v_sb[:, kb, 2 * p + h, :],
                                     start=(kb == 0), stop=(kb == qb))
            o_sb = spool.tile([128, 128], BF16, tag="osb", name="osb")
            rec = spool.tile([128, 2], F32, tag="rec", name="rec")
            nc.vector.reciprocal(rec, pvps[:, :, 64:65].rearrange("p a b -> p (a b)"))
            for h in range(2):
                nc.vector.tensor_scalar_mul(o_sb[:, h * 64:(h + 1) * 64],
                                            pvps[:, h, 0:64], rec[:, h:h + 1])
            nc.sync.dma_start_transpose(
                out=oT[p][:, qb * 128:(qb + 1) * 128], in_=o_sb)

        def att_pv(p, tau):
            for qb in range(4 * tau, 4 * tau + 4):
                pv_step(p, qb)

        def proj_step(t16, ps_proj):
            o_out = opool.tile([128, C], BF16, tag="oout", name="oout")
            ps = ps_proj.tile([128, C], F32, tag="proj", name="psproj")
            for half in range(2):
                for dc in range(2):
                    nc.tensor.matmul(
                        ps[:, half * 512:(half + 1) * 512],
                        oT[dc][:, t16 * 128:(t16 + 1) * 128],
                        wp_sb[:, dc, half * 512:(half + 1) * 512],
                        start=(dc == 0), stop=(dc == 1))
            if t16 % 2 == 0:
                nc.vector.tensor_copy(o_out, ps)
            else:
                nc.scalar.copy(o_out, ps)
            nc.sync.dma_start(out=d["out"][t16 * 128:(t16 + 1) * 128, :],
                              in_=o_out)

        # ==================== pipelined emission ====================
        atp1 = [None]
        qk_step(0, 0)
        permute(0, 0, 512)
        qk_step(0, 1)
        permute(0, 512, 1024)
        att_sc(0, 0)
        qk_step(0, 2)
        att_sc(0, 1)
        qk_step(0, 3)
        permute(0, 1024, 2048)
        import os as _os
        _dq = float(_os.environ.get("D_QK1", "30")) / 1000.0
        _dv = float(_os.environ.get("D_V", "44")) / 1000.0
        with tc.tile_wait_until(_dq):
            qk_step(1, 0)
        att_sc(0, 2)
        with tc.tile_wait_until(_dq + 0.004):
            qk_step(1, 1)
        att_sc(0, 3)
        with tc.tile_wait_until(_dq + 0.008):
            qk_step(1, 2)
            qk_step(1, 3)
        permute(1, 0, 2048)
        _cmw.close()          # ck/sk/wqk/rot dead
        with tc.tile_wait_until(_dv):
            for kc in range(0, 16):
                v_step(kc)

        # x and the qkv psum are dead; free for pair-1 at tiles + proj psum
        _cm.close()
        _cmx.close()
        atp1[0] = _cm.enter_context(tc.tile_pool(name="atp1", bufs=1))

        att_sc(1, 0)          # atp1 tags, reuse x region
        att_sc(1, 1)
        att_pv(0, 0)
        att_pv(0, 1)
        att_sc(1, 2)          # atp1 tags too -> independent of pair-0 pv
        att_pv(0, 2)
        att_pv(0, 3)
        att_pv(1, 0)
        att_sc(1, 3)          # shared tags (WAR-safe after att_pv(0, 3))
        _cmsc.close()         # scores psum banks -> proj
        ps_proj = _cm.enter_context(
            tc.tile_pool(name="ps_proj", bufs=3, space="PSUM"))
        att_pv(1, 1)
        for t16 in range(0, 8):
            proj_step(t16, ps_proj)
        att_pv(1, 2)
        for t16 in range(8, 12):
            proj_step(t16, ps_proj)
        att_pv(1, 3)
        for t16 in range(12, 16):
            proj_step(t16, ps_proj)

        if dbg is not None:
            nc.sync.dma_start(out=dbg["qr0"], in_=qr[0])
            nc.sync.dma_start(out=dbg["kr0"], in_=kr[0])
            nc.sync.dma_start(out=dbg["v"],
                              in_=v_sb.rearrange("p a b c -> p (a b c)"))
            nc.sync.dma_start(
                out=dbg["at0"],
                in_=atp_tiles[1][0][:, 0, :, :].rearrange("p a b -> p (a b)"))
            nc.sync.dma_start(
                out=dbg["at5"],
                in_=atp_tiles[1][2][:, 1, :, :].rearrange("p a b -> p (a b)"))
            nc.sync.dma_start(out=dbg["oT0"], in_=oT[0])
            nc.sync.dma_start(out=dbg["oT1"], in_=oT[1])


def _host_prep(hidden_states, cos, sin, qkv_w, qkv_b, proj_w):
    cos_rep = np.tile(np.ascontiguousarray(cos.T), (4, 1))
    sin_rep = np.tile(np.ascontiguousarray(sin.T), (4, 1))
    ck = cos_rep.astype(BF)
    sk = sin_rep.astype(BF)
    mask01 = (np.arange(128)[:, None] <= np.arange(128)[None, :]).astype(BF)

    in_maps = []
    for c in range(NCORES):
        b = c // CORES_PER_B
        h0 = (c % CORES_PER_B) * HPC
        heads = list(range(h0, h0 + HPC))
        vrows = [h * D + dd for h in heads for dd in range(D)]
        m = dict(
            xT=np.ascontiguousarray(hidden_states[b].T).astype(BF),
            wv_t=np.ascontiguousarray(
                qkv_w[2 * H * D:3 * H * D][vrows].T).astype(BF),
            bv=np.ascontiguousarray(qkv_b[2 * H * D:3 * H * D][vrows]),
            wproj_t=np.ascontiguousarray(proj_w[:, vrows].T).astype(BF),
            cos_k=ck, sin_k=sk, mask01=mask01,
        )
        qw = qkv_w[0 * H * D:1 * H * D]
        kw = qkv_w[1 * H * D:2 * H * D]
        qb_ = qkv_b[0 * H * D:1 * H * D]
        kb_ = qkv_b[1 * H * D:2 * H * D]
        for p in range(2):
            hA, hB = h0 + 2 * p, h0 + 2 * p + 1
            for ab in range(2):
                # rows: q-hA dims, q-hB dims, k-hA dims, k-hB dims (32 each),
                # dims = even (ab=0) or odd (ab=1) rotary positions
                dims = [2 * j + ab for j in range(D // 2)]
                rows_q = [hA * D + dd for dd in dims] + \
                         [hB * D + dd for dd in dims]
                rows_k = rows_q
                wtile = np.concatenate(
                    [qw[rows_q] * SCALE, kw[rows_k]], axis=0)   # (128, C)
                btile = np.concatenate(
                    [qb_[rows_q] * SCALE, kb_[rows_k]], axis=0)  # (128,)
                wt = wtile.T.reshape(CCH, 128, 128).transpose(1, 0, 2)
                m[f"wqk{p}{ab}"] = np.ascontiguousarray(
                    wt.reshape(128, CCH * 128)).astype(BF)
                m[f"bqk{p}{ab}"] = np.ascontiguousarray(btile)
        in_maps.append(m)
    return in_maps


def kernel(hidden_states, cos, sin, qkv_w, qkv_b, proj_w, proj_b):
    hidden_states = np.asarray(hidden_states, dtype=np.float32)
    cos = np.asarray(cos, dtype=np.float32)
    sin = np.asarray(sin, dtype=np.float32)
    qkv_w = np.asarray(qkv_w, dtype=np.float32)
    qkv_b = np.asarray(qkv_b, dtype=np.float32)
    proj_w = np.asarray(proj_w, dtype=np.float32)
    proj_b = np.asarray(proj_b, dtype=np.float32)

    if "nc" not in _compiled:
        _compiled["nc"] = _build()
    nc = _compiled["nc"]

    in_maps = _host_prep(hidden_states, cos, sin, qkv_w, qkv_b, proj_w)
    res = run_bass_kernel_spmd(nc, in_maps, core_ids=list(range(NCORES)))
    outs = [np.asarray(res.results[c]["out"], dtype=np.float32)
            for c in range(NCORES)]
    final = np.empty((B, T, C), np.float32)
    for b in range(B):
        acc = outs[b * CORES_PER_B].copy()
        for i in range(1, CORES_PER_B):
            acc += outs[b * CORES_PER_B + i]
        final[b] = acc + proj_b[None, :]
    return final
